# revision 24
# baseline (speedup 1.0000x reference)
"""Trainium2 Bass kernel for nn_CrossAttention_22600117911813.

Per-chunk cross attention:
  y = LN(e) -> Q; h -> K,V;  softmax(Q K^T / 8) V -> out proj + residual.

Sharding: data-parallel over the b*c chunk dim (64 chunks) across 8 cores,
8 chunks per core; projection weights replicated.

Host-side preprocessing (pure layout / algebraic folds, all exact):
  - ln_g/ln_b folded into wq/bq          (z*g+b) @ wq = z @ (g*wq) + b@wq
  - bk dropped                            (adds a per-(i,h) constant to every
                                           j logit -> softmax invariant)
  - bv folded into bo                     o = p@(v0 + 1*bv) -> + bv @ wo
  - h transposed to h^T (d_model major)   so K^T / V projections need no
                                           on-device transpose

On-device dataflow per chunk (all matmuls in float32r at >=256 free dim):
  e [128,1024] -> LN stats (bn_stats) -> z -> PE-transpose -> z^T
  Q^T[hd,256]  = wq^T z^T   (both neighbors batched: N=256)
  K^T[hd,j]    = wk^T h^T   (4 chunks batched: N=256, phase 0)
  V[j,hd]      = h^T^T wv   (2 chunks stacked: M=128, phase 0)
  S^T[j,256]   = K_h Q_h^T  per head, head pairs stacked in PSUM
  P = exp(S/8) (ACT, no max-sub: logits bounded ~ +-6)
  colsum via ones-matmul, broadcast via select-matmul, 1/x on DVE
  O^T[d,256]   = V_h^T P^T  per head -> normalized on PSUM->SBUF copy
  y[128,1024]  = O wo (+ bo via rank-1 ones matmul) + e (residual fused
                 into the PSUM->SBUF copy on DVE)
"""

import numpy as np

import concourse.bass as bass
import concourse.mybir as mybir
import concourse.tile as tile
from concourse import bacc
from concourse.bass_utils import run_bass_kernel_spmd

F32 = mybir.dt.float32
F32R = mybir.dt.float32r
AF = mybir.ActivationFunctionType

# problem shapes (hardcoded per contract)
B, C, NN, I, D = 2, 32, 2, 128, 1024
J = 64
NH, DK = 16, 64
EPS = 1e-5
SCALE = 1.0 / np.sqrt(DK)

CORES = 8
CPC = (B * C) // CORES  # chunks per core = 8
KT_ = D // 128          # 8 d_model tiles
MT = D // 128           # 8 hd tiles

USE_F32R = True
MDT = F32R if USE_F32R else F32  # matmul operand dtype


def _r(ap):
    if not USE_F32R or ap.dtype == F32R:
        return ap
    return ap.bitcast(F32R)


def _f32(ap):
    # fp32r matmuls require dst start_partition == 0; matmuls that write to
    # PSUM partition 64 must run as plain fp32 (operands viewed as fp32).
    return ap.bitcast(F32) if ap.dtype == F32R else ap


def _build_body(nc, tc, ctx, aps):
    e_d, ht_d, wq_d, wk_d, wv_d, wo_d, bq_d, consts_d, y_d = aps

    const = ctx.enter_context(tc.tile_pool(name="const", bufs=1))
    wpool = ctx.enter_context(tc.tile_pool(name="wpool", bufs=2))
    kvpool = ctx.enter_context(tc.tile_pool(name="kv", bufs=1))

    # ---- constant tiles (fp32r; filled from the packed consts input) ----
    ident = const.tile([128, 128], MDT)
    ones2 = const.tile([128, 2], MDT)
    sel2 = const.tile([2, 128], MDT)
    onesc = const.tile([1, 128], MDT)
    bo_sb = const.tile([1, D], MDT)
    eps_t = const.tile([128, 1], F32)
    nc.vector.memset(eps_t, EPS)
    bq_sb = const.tile([128, MT], F32)
    nc.sync.dma_start(out=bq_sb, in_=bq_d.rearrange("(m p) -> p m", p=128))

    # ---- persistent K^T / V for all chunks ----
    kt_sb = kvpool.tile([128, MT, CPC, J], MDT)   # [hd%128, m, c, j]
    # V duplicated into both partition halves so the AV matmul's lhsT can
    # match the rhs (P) base partition for either head parity.
    v_sb = kvpool.tile([128, CPC, D], MDT)        # [dup(j), c, hd]

    # ---- phase 0: consts, weights, K^T and V projections ----
    # fp32r matmul operands must be produced by a compute op that rounds to
    # fp32r (walrus birverifier); DMA does not round, and the DMA pseudo-inst
    # supports a single semaphore wait — so all DMAs land in F32 staging
    # tiles whose only other accessor is the converting scalar-engine copy,
    # and each distinct transfer shape gets its own stage tag (same-shape
    # DMAs share one SWDGE queue, keeping slot-recycle waits at one).
    # Staging pools are scoped to phase 0 so the SBUF is reclaimed for the
    # per-chunk pipeline pools.
    with tc.tile_pool(name="stage", bufs=1) as stpool, \
         tc.tile_pool(name="hpool", bufs=1) as hpool, \
         tc.tile_pool(name="ps0", bufs=3, space="PSUM") as ps0:
        cst = stpool.tile([128, 1410], F32, tag="cst", bufs=1)
        nc.gpsimd.dma_start(out=cst, in_=consts_d)
        nc.scalar.activation(out=ident, in_=cst[:, 0:128], func=AF.Copy)
        nc.scalar.activation(out=sel2, in_=cst[0:2, 128:256], func=AF.Copy)
        nc.scalar.activation(out=bo_sb, in_=cst[0:1, 256:1280], func=AF.Copy)
        nc.scalar.activation(out=onesc, in_=cst[0:1, 1280:1408], func=AF.Copy)
        nc.scalar.activation(out=ones2, in_=cst[:, 1408:1410], func=AF.Copy)

        def staged(dst, src_ap, tag, shape):
            stg = stpool.tile(shape, F32, tag=tag, bufs=2)
            view = stg[:dst.partition_size(), :dst.free_size()]
            nc.sync.dma_start(out=view, in_=src_ap)
            flat = dst if dst.ndim == 2 else dst.rearrange("p a b -> p (a b)")
            nc.scalar.activation(out=flat, in_=view, func=AF.Copy)

        def load_weight(w_d):
            w_sb = wpool.tile([128, KT_, D], MDT, tag="w")
            for k in range(KT_):
                staged(w_sb[:, k, :], w_d[k * 128:(k + 1) * 128, :],
                       "wstage", [128, 1024])
            return w_sb

        wk_sb = load_weight(wk_d)
        wv_sb = load_weight(wv_d)

        ht_sb = hpool.tile([128, KT_, CPC, J], MDT)
        for k in range(KT_):
            staged(ht_sb[:, k, :, :],
                   ht_d[:, k * 128:(k + 1) * 128, :].rearrange(
                       "c p j -> p c j"),
                   "hstage", [128, CPC * J])

        for m in range(MT):
            pt = ps0.tile([128, 512], F32, tag="ps0")
            for g2 in range(2):
                for k in range(KT_):
                    nc.tensor.matmul(
                        pt[:, g2 * 256:(g2 + 1) * 256],
                        lhsT=_r(wk_sb[:, k, m * 128:(m + 1) * 128]),
                        rhs=_r(ht_sb[:, k, 4 * g2:4 * g2 + 4, :].rearrange(
                            "p a b -> p (a b)")),
                        start=(k == 0), stop=(k == KT_ - 1),
                    )
            nc.scalar.activation(
                out=kt_sb[:, m, :, :].rearrange("p a b -> p (a b)"), in_=pt,
                func=AF.Copy)

        for pr in range(CPC // 2):
            for half in range(2):
                pv = ps0.tile([128, 512], F32, tag="ps0")
                for k in range(KT_):
                    nc.tensor.matmul(
                        pv,
                        lhsT=_r(ht_sb[:, k, 2 * pr:2 * pr + 2, :].rearrange(
                            "p a b -> p (a b)")),
                        rhs=_r(wv_sb[:, k, half * 512:(half + 1) * 512]),
                        start=(k == 0), stop=(k == KT_ - 1),
                    )
                for ci in range(2):
                    for par in range(2):
                        nc.scalar.activation(
                            out=v_sb[par * 64:(par + 1) * 64, 2 * pr + ci,
                                     half * 512:(half + 1) * 512],
                            in_=pv[ci * 64:(ci + 1) * 64, :], func=AF.Copy)

        # weight slots recycle: wq -> wk's slot, wo -> wv's slot
        wq_sb = load_weight(wq_d)
        wo_sb = load_weight(wo_d)

    # ---- per-chunk pipeline pools ----
    epool = ctx.enter_context(tc.tile_pool(name="epool", bufs=3))
    zpool = ctx.enter_context(tc.tile_pool(name="zpool", bufs=1))
    ztpool = ctx.enter_context(tc.tile_pool(name="ztpool", bufs=1))
    qtpool = ctx.enter_context(tc.tile_pool(name="qtpool", bufs=2))
    ppool = ctx.enter_context(tc.tile_pool(name="ppool", bufs=1))
    invpool = ctx.enter_context(tc.tile_pool(name="invpool", bufs=1))
    otpool = ctx.enter_context(tc.tile_pool(name="otpool", bufs=2))
    ypool = ctx.enter_context(tc.tile_pool(name="ypool", bufs=2))
    small = ctx.enter_context(tc.tile_pool(name="small", bufs=4))
    psA = ctx.enter_context(tc.tile_pool(name="psA", bufs=3, space="PSUM"))
    psS = ctx.enter_context(tc.tile_pool(name="psS", bufs=2, space="PSUM"))
    psY = ctx.enter_context(tc.tile_pool(name="psY", bufs=2, space="PSUM"))

    for c in range(CPC):
        # -------- load e, layernorm, transpose z --------
        e_t = []
        for n in range(NN):
            et = epool.tile([128, D], F32, tag="e")
            nc.sync.dma_start(out=et, in_=e_d[c, n])
            e_t.append(et)

        zt_sb = ztpool.tile([128, KT_, 2 * I], MDT, tag="zt")
        for n in range(NN):
            st = small.tile([128, 2, 6], F32, tag="st")
            nc.vector.bn_stats(out=st[:, 0, :], in_=e_t[n][:, 0:512])
            nc.vector.bn_stats(out=st[:, 1, :], in_=e_t[n][:, 512:1024])
            mv = small.tile([128, 2], F32, tag="mv")
            nc.vector.bn_aggr(out=mv, in_=st)
            rstd = small.tile([128, 1], F32, tag="rstd")
            nc.scalar.activation(out=rstd, in_=mv[:, 1:2], func=AF.Sqrt,
                                 bias=eps_t)
            nc.vector.reciprocal(out=rstd, in_=rstd)
            z = zpool.tile([128, D], MDT, tag="z")
            with nc.allow_low_precision(reason="fp32r rounding of LN out"):
                nc.vector.tensor_scalar(
                    out=z, in0=e_t[n], scalar1=mv[:, 0:1], scalar2=rstd,
                    op0=mybir.AluOpType.subtract, op1=mybir.AluOpType.mult)
            for kq in range(0, KT_, 4):
                zp = psA.tile([128, 512], F32, tag="psA")
                for q in range(4):
                    nc.tensor.transpose(
                        out=_r(zp[:, q * 128:(q + 1) * 128]),
                        in_=_r(z[:, (kq + q) * 128:(kq + q + 1) * 128]),
                        identity=_r(ident),
                    )
                nc.scalar.activation(
                    out=zt_sb[:, kq:kq + 4, n * I:(n + 1) * I],
                    in_=zp.rearrange("p (q i) -> p q i", q=4), func=AF.Copy)

        # -------- Q^T projection (both neighbors: N=256) --------
        qt_sb = qtpool.tile([128, MT, 2 * I], MDT, tag="qt")
        for mp in range(MT // 2):
            qp = psA.tile([128, 512], F32, tag="psA")
            for mi in range(2):
                m = 2 * mp + mi
                for k in range(KT_):
                    nc.tensor.matmul(
                        qp[:, mi * 256:(mi + 1) * 256],
                        lhsT=_r(wq_sb[:, k, m * 128:(m + 1) * 128]),
                        rhs=_r(zt_sb[:, k, :]),
                        start=(k == 0), stop=(k == KT_ - 1),
                    )
            for mi in range(2):
                m = 2 * mp + mi
                nc.scalar.activation(
                    out=qt_sb[:, m, :], in_=qp[:, mi * 256:(mi + 1) * 256],
                    func=AF.Identity, bias=bq_sb[:, m:m + 1])

        # -------- scores, softmax (no max-sub), colsum+bcast --------
        p_sb = ppool.tile([128, NH // 2, 2 * I], MDT, tag="p")
        inv_sb = invpool.tile([128, NH // 2, 2 * I], F32, tag="inv")
        for hp in range(NH // 2):
            sp = psS.tile([128, 2, 2 * I], F32, tag="psS")
            for par in range(2):
                cvt = _r if par == 0 else _f32
                nc.tensor.matmul(
                    sp[par * 64:(par + 1) * 64, 0, :],
                    lhsT=cvt(kt_sb[par * 64:(par + 1) * 64, hp, c, :]),
                    rhs=cvt(qt_sb[par * 64:(par + 1) * 64, hp, :]),
                    start=True, stop=True,
                )
            nc.scalar.activation(out=p_sb[:, hp, :], in_=sp[:, 0, :],
                                 func=AF.Exp, scale=float(SCALE))
            # per-head column sums (over j = partitions) via ones matmul
            nc.tensor.matmul(
                sp[0:2, 1, :], lhsT=_r(ones2), rhs=_r(p_sb[:, hp, :]),
                start=True, stop=True,
            )
            csi = small.tile([2, 2 * I], MDT, tag="csi", bufs=1)
            with nc.allow_low_precision(reason="fp32r rounding of 1/colsum"):
                nc.vector.reciprocal(out=csi, in_=sp[0:2, 1, :])
            # broadcast [2,256] -> [128,256] via select matmul
            bp = psA.tile([128, 512], F32, tag="psA")
            nc.tensor.matmul(
                bp[:, 0:256], lhsT=_r(sel2), rhs=_r(csi),
                start=True, stop=True,
            )
            nc.scalar.activation(out=inv_sb[:, hp, :], in_=bp[:, 0:256],
                                 func=AF.Copy)

        # -------- attention output O^T, normalized on copy-out --------
        ot_sb = otpool.tile([128, MT, 2 * I], MDT, tag="ot")
        for hpp in range(NH // 4):
            op_ = psA.tile([128, 512], F32, tag="psA")
            for hi in range(2):
                hp = 2 * hpp + hi
                for par in range(2):
                    h = 2 * hp + par
                    cvt = _r if par == 0 else _f32
                    nc.tensor.matmul(
                        op_[par * 64:(par + 1) * 64, hi * 256:(hi + 1) * 256],
                        lhsT=cvt(v_sb[par * 64:(par + 1) * 64, c,
                                      h * DK:(h + 1) * DK]),
                        rhs=cvt(p_sb[par * 64:(par + 1) * 64, hp, :]),
                        start=True, stop=True,
                    )
                nc.vector.tensor_mul(
                    ot_sb[:, hp, :], op_[:, hi * 256:(hi + 1) * 256],
                    inv_sb[:, hp, :])

        # -------- output projection + bias + residual --------
        for n in range(NN):
            y_sb = ypool.tile([128, D], F32, tag="y")
            for half in range(2):
                yp = psY.tile([128, 512], F32, tag="psY")
                nc.tensor.matmul(
                    yp, lhsT=_r(onesc),
                    rhs=_r(bo_sb[0:1, half * 512:(half + 1) * 512]),
                    start=True, stop=False,
                )
                for k in range(MT):
                    nc.tensor.matmul(
                        yp,
                        lhsT=_r(ot_sb[:, k, n * I:(n + 1) * I]),
                        rhs=_r(wo_sb[:, k, half * 512:(half + 1) * 512]),
                        start=False, stop=(k == MT - 1),
                    )
                nc.vector.tensor_add(
                    y_sb[:, half * 512:(half + 1) * 512], yp,
                    e_t[n][:, half * 512:(half + 1) * 512])
            nc.sync.dma_start(out=y_d[c, n], in_=y_sb)


def make_program():
    nc = bacc.Bacc(trn_type="TRN2", target_bir_lowering=False, debug=False)
    e_d = nc.declare_dram_parameter("e", [CPC, NN, I, D], F32, isOutput=False).ap()
    ht_d = nc.declare_dram_parameter("hT", [CPC, D, J], F32, isOutput=False).ap()
    wq_d = nc.declare_dram_parameter("wq", [D, D], F32, isOutput=False).ap()
    wk_d = nc.declare_dram_parameter("wk", [D, D], F32, isOutput=False).ap()
    wv_d = nc.declare_dram_parameter("wv", [D, D], F32, isOutput=False).ap()
    wo_d = nc.declare_dram_parameter("wo", [D, D], F32, isOutput=False).ap()
    bq_d = nc.declare_dram_parameter("bq", [D], F32, isOutput=False).ap()
    consts_d = nc.declare_dram_parameter("consts", [128, 1410], F32,
                                         isOutput=False).ap()
    y_d = nc.declare_dram_parameter("y", [CPC, NN, I, D], F32, isOutput=True).ap()

    from contextlib import ExitStack
    with tile.TileContext(nc) as tc, ExitStack() as ctx:
        _build_body(nc, tc, ctx,
                    (e_d, ht_d, wq_d, wk_d, wv_d, wo_d, bq_d, consts_d, y_d))
    nc.compile()
    return nc


def prepare_shards(inputs):
    """Host-side folds + sharding. Returns per-core input maps."""
    e = np.ascontiguousarray(np.asarray(inputs["e"], dtype=np.float32))
    h = np.asarray(inputs["h"], dtype=np.float32)
    wq = np.asarray(inputs["wq"], dtype=np.float32)
    bq = np.asarray(inputs["bq"], dtype=np.float32)
    wk = np.asarray(inputs["wk"], dtype=np.float32)
    wv = np.asarray(inputs["wv"], dtype=np.float32)
    bv = np.asarray(inputs["bv"], dtype=np.float32)
    wo = np.asarray(inputs["wo"], dtype=np.float32)
    bo = np.asarray(inputs["bo"], dtype=np.float32)
    g = np.asarray(inputs["ln_g"], dtype=np.float32)
    b_ = np.asarray(inputs["ln_b"], dtype=np.float32)

    wq_eff = np.ascontiguousarray(g[:, None] * wq)
    bq_eff = (bq + b_ @ wq).astype(np.float32)
    bo_eff = (bo + bv @ wo).astype(np.float32)

    consts = np.zeros((128, 1410), np.float32)
    consts[:, 0:128] = np.eye(128, dtype=np.float32)       # transpose identity
    consts[0, 128 + 0:128 + 64] = 1.0                      # sel2 row 0
    consts[1, 128 + 64:128 + 128] = 1.0                    # sel2 row 1
    consts[0, 256:1280] = bo_eff                           # output bias row
    consts[0, 1280:1408] = 1.0                             # onesc
    consts[0:64, 1408] = 1.0                               # ones2 col 0
    consts[64:128, 1409] = 1.0                             # ones2 col 1

    e_flat = e.reshape(B * C, NN, I, D)
    ht = np.ascontiguousarray(
        h.reshape(B * C, J, D).transpose(0, 2, 1))  # [bc, D, J]

    in_maps = []
    for core in range(CORES):
        sl = slice(core * CPC, (core + 1) * CPC)
        in_maps.append({
            "e": np.ascontiguousarray(e_flat[sl]),
            "hT": np.ascontiguousarray(ht[sl]),
            "wq": wq_eff, "wk": wk, "wv": wv, "wo": wo,
            "bq": bq_eff, "consts": consts,
        })
    return in_maps


_PROGRAM = None


def kernel(**inputs):
    global _PROGRAM
    if _PROGRAM is None:
        _PROGRAM = make_program()
    nc = _PROGRAM
    in_maps = prepare_shards(inputs)
    res = run_bass_kernel_spmd(nc, in_maps, list(range(CORES)))
    y = np.concatenate([r["y"][None] for r in res.results], axis=0)
    return y.reshape(B, C, NN, I, D).astype(np.float32)
